# revision 1
# baseline (speedup 1.0000x reference)
"""Trainium2 Bass kernel for ContextAwareRegionalAttentionNetwork.

Computes, for B=4 images of [C=2048, 80, 80] features and R=2000 ROIs:
  roi_mean[r, c]  = mean of features[b_r, c] over the ROI window
  pooled[r]       = concat(roi_mean[r], gmean[b_r])            # [2C]
  out[0, r]       = softplus(W2 @ tanh(W1 @ pooled[r] + b1) + b2)

Strategy (8 NeuronCores, channel-sharded):
  - core k owns channels [256k, 256k+256) of every image (26 MB of features).
  - per 128-channel plane tile: masked tensor_tensor_scan (x-cumsum) ->
    strided-copy free-dim transpose (split gpsimd/scalar engines) ->
    masked scan (y-cumsum) = 2D summed-area table (SAT).
  - ap_gather pulls the 4 SAT corners for each ROI of that image;
    win = g0-g1-g2+g3 scaled by 1/area, matmul with the core's W1 slice
    accumulates pre-activations for all ROIs in PSUM.
  - global-context term: SAT total sum -> [64, B] matmul -> per-ROI gather.
  - AllReduce [64, R] over the 8 cores sums the channel partials, then
    tanh / W2 / softplus run on-device; host just unpermutes columns.

ROIs are sorted by batch index on the host (tiny [2000, 5] tensor); the
final [1, R] is unpermuted back. All heavy data (features) is processed
on-device.
"""

import numpy as np
from contextlib import ExitStack

import concourse.bass as bass
import concourse.tile as tile
from concourse import bacc, mybir
from concourse.bass_utils import run_bass_kernel_spmd

f32 = mybir.dt.float32
i16 = mybir.dt.int16

B, C, H, W = 4, 2048, 80, 80
R = 2000
SCALE = 0.03125
N = H * W                  # 6400 per plane
CPC = C // 8               # 256 channels per core
ZOFF = N                   # zero-element offset inside the SAT tile
SATW = N + 16              # SAT tile free width (16 zero slots)
NCORES = 8


def _wrap_idx(flat, channels):
    """Wrap a flat index list for ap_gather: idx k lives at partition k%16
    (replicated across each 16-partition group), free slot k//16."""
    flat = np.asarray(flat, np.int16)
    n = len(flat)
    assert n % 16 == 0
    cols = n // 16
    out = np.zeros((channels, cols), np.int16)
    grid = flat.reshape(cols, 16).T          # [16, cols]
    for g in range(channels // 16):
        out[g * 16:(g + 1) * 16, :] = grid
    return out


def _host_prep(rois):
    """Decode ROIs exactly like the reference, sort by image, build gather
    indices / reciprocal areas / column mapping."""
    rois = np.asarray(rois, np.float32)
    b = rois[:, 0].astype(np.int32)
    coords = np.round(rois[:, 1:] * np.float32(SCALE)).astype(np.int32)
    x1, y1, x2, y2 = coords[:, 0], coords[:, 1], coords[:, 2], coords[:, 3]
    rw = np.maximum(x2 - x1 + 1, 1)
    rh = np.maximum(y2 - y1 + 1, 1)
    hs = np.clip(y1, 0, H)
    he = np.clip(y1 + rh, 0, H)
    ws = np.clip(x1, 0, W)
    we = np.clip(x1 + rw, 0, W)
    area = ((he - hs) * (we - ws)).astype(np.float32)
    empty = (he <= hs) | (we <= ws)
    recip = np.where(empty, 0.0, 1.0 / np.maximum(area, 1.0)).astype(np.float32)

    order = np.argsort(b, kind="stable")
    groups = [order[b[order] == img] for img in range(B)]
    rbp = [(len(g) + 3) // 4 * 4 for g in groups]          # pad to mult of 4
    while sum(rbp) % 16:                                   # idx wrap needs %16
        rbp[-1] += 4
    offs = np.concatenate([[0], np.cumsum(rbp)]).astype(int)
    rp = int(offs[-1])
    assert rp % 4 == 0

    def corner(yy, xx):
        # SAT interior value S[y', x'] sits at (x'-1)*80 + (y'-1); row/col 0
        # of the padded SAT is identically zero -> dedicated zero slot.
        return np.where((yy == 0) | (xx == 0), ZOFF,
                        (xx - 1) * W + (yy - 1)).astype(np.int16)

    idx_imgs = []
    recip_sorted = np.zeros(rp, np.float32)
    bcol = np.full(rp, 4, np.int16)          # pads -> zero column of gsb
    colmap = np.zeros(R, np.int64)
    for img in range(B):
        g = groups[img]
        npad = rbp[img] - len(g)
        colmap[g] = offs[img] + np.arange(len(g))
        recip_sorted[offs[img]:offs[img] + len(g)] = recip[g]
        bcol[offs[img]:offs[img] + len(g)] = np.where(empty[g], 4, img)
        c00 = corner(he[g], we[g])
        c01 = corner(hs[g], we[g])
        c10 = corner(he[g], ws[g])
        c11 = corner(hs[g], ws[g])
        zpad = np.full(npad, ZOFF, np.int16)
        flat = np.concatenate([c00, zpad, c01, zpad, c10, zpad, c11, zpad])
        idx_imgs.append(_wrap_idx(flat, 128))

    idx_g = _wrap_idx(bcol, 64)
    recip_rep = np.broadcast_to(recip_sorted, (128, rp)).copy()
    return idx_imgs, idx_g, recip_rep, rbp, offs, rp, colmap


def _build(rbp, offs, rp):
    nc = bacc.Bacc("TRN2", target_bir_lowering=False, debug=False,
                   num_devices=NCORES)
    feat_d = nc.dram_tensor("feat", [B * CPC, N], f32, kind="ExternalInput").ap()
    w1a_d = nc.dram_tensor("w1a", [CPC, 64], f32, kind="ExternalInput").ap()
    w1g_d = nc.dram_tensor("w1g", [CPC, 64], f32, kind="ExternalInput").ap()
    recip_d = nc.dram_tensor("recip", [128, rp], f32, kind="ExternalInput").ap()
    idx_d = [nc.dram_tensor(f"idx{img}", [128, rbp[img] // 4], i16,
                            kind="ExternalInput").ap() for img in range(B)]
    idxg_d = nc.dram_tensor("idxg", [64, rp // 16], i16, kind="ExternalInput").ap()
    b1_d = nc.dram_tensor("b1", [64, 1], f32, kind="ExternalInput").ap()
    w2t_d = nc.dram_tensor("w2t", [64, 1], f32, kind="ExternalInput").ap()
    b2_d = nc.dram_tensor("b2", [1, 1], f32, kind="ExternalInput").ap()
    out_d = nc.dram_tensor("out", [1, rp], f32, kind="ExternalOutput").ap()
    dbgsat_d = nc.dram_tensor("dbgsat", [128, SATW], f32, kind="ExternalOutput").ap()
    dbgpre_d = nc.dram_tensor("dbgpre", [64, rp], f32, kind="ExternalOutput").ap()
    dbgwin_d = nc.dram_tensor("dbgwin", [128, rp], f32, kind="ExternalOutput").ap()
    dbggx_d = nc.dram_tensor("dbggx", [64, rp], f32, kind="ExternalOutput").ap()
    dbgpm_d = nc.dram_tensor("dbgpm", [64, rp], f32, kind="ExternalOutput").ap()

    HN = N // 2                                # 3200: scan half width
    with tile.TileContext(nc) as tc, ExitStack() as ctx:
        const = ctx.enter_context(tc.tile_pool(name="const", bufs=1))
        fpool = ctx.enter_context(tc.tile_pool(name="feat", bufs=2))
        spool = ctx.enter_context(tc.tile_pool(name="sat", bufs=2))
        gpool = ctx.enter_context(tc.tile_pool(name="gout", bufs=2))
        wpool = ctx.enter_context(tc.tile_pool(name="win", bufs=2))
        epool = ctx.enter_context(tc.tile_pool(name="epi", bufs=1))
        pmain = ctx.enter_context(tc.tile_pool(name="pm", bufs=1, space="PSUM"))
        pgp = ctx.enter_context(tc.tile_pool(name="pg", bufs=1, space="PSUM"))
        pw2 = ctx.enter_context(tc.tile_pool(name="pw2", bufs=2, space="PSUM"))
        dram = ctx.enter_context(tc.tile_pool(name="dram", bufs=1, space="DRAM"))

        # constants
        mask = const.tile([128, HN], f32)
        nc.vector.memset(mask[:], 1.0)
        nc.vector.memset(mask[:].rearrange("p (r w) -> p r w", w=W)[:, :, 0:1], 0.0)
        recip = const.tile([128, rp], f32)
        nc.sync.dma_start(recip[:], recip_d[:])
        w1a = [const.tile([128, 64], f32, tag=f"w1a{cb}", name=f"w1a{cb}") for cb in range(2)]
        w1g = [const.tile([128, 64], f32, tag=f"w1g{cb}", name=f"w1g{cb}") for cb in range(2)]
        for cb in range(2):
            nc.sync.dma_start(w1a[cb][:], w1a_d[cb * 128:(cb + 1) * 128, :])
            nc.sync.dma_start(w1g[cb][:], w1g_d[cb * 128:(cb + 1) * 128, :])
        idxs = []
        for img in range(B):
            t = const.tile([128, rbp[img] // 4], i16, tag=f"idx{img}", name=f"idxt{img}")
            nc.sync.dma_start(t[:], idx_d[img][:])
            idxs.append(t)
        idxg = const.tile([64, rp // 16], i16)
        nc.sync.dma_start(idxg[:], idxg_d[:])
        b1t = const.tile([64, 1], f32)
        nc.sync.dma_start(b1t[:], b1_d[:])
        w2t = const.tile([64, 1], f32)
        nc.sync.dma_start(w2t[:], w2t_d[:])
        b2t = const.tile([1, 1], f32)
        nc.sync.dma_start(b2t[:], b2_d[:])

        psum_main = pmain.tile([64, rp], f32)
        psum_g = pgp.tile([64, B], f32)

        mm = mybir.AluOpType.mult
        add = mybir.AluOpType.add
        sub = mybir.AluOpType.subtract

        for img in range(B):
            rb = rbp[img]
            for cb in range(2):
                row0 = img * CPC + cb * 128
                ft = fpool.tile([128, N], f32)
                nc.sync.dma_start(ft[:], feat_d[row0:row0 + 128, :])
                # x-cumsum (rows of 80), in place, two halves
                for h in range(2):
                    sl = ft[:, h * HN:(h + 1) * HN]
                    nc.vector.tensor_tensor_scan(
                        out=sl, data0=mask[:], data1=sl, initial=0.0,
                        op0=mm, op1=add)
                sat = spool.tile([128, SATW], f32)
                # transpose rowcum into x-major order; split across engines
                src = ft[:].rearrange("p (y x) -> p x y", x=W)
                dst = sat[:, 0:N].rearrange("p (x y) -> p x y", y=H)
                nc.gpsimd.tensor_copy(dst[:, 0:W // 2, :], src[:, 0:W // 2, :])
                nc.gpsimd.tensor_copy(dst[:, W // 2:, :], src[:, W // 2:, :])
                # y-cumsum in place -> SAT (x-major: S[y',x'] at (x'-1)*80+(y'-1))
                for h in range(2):
                    sl = sat[:, h * HN:(h + 1) * HN]
                    nc.vector.tensor_tensor_scan(
                        out=sl, data0=mask[:], data1=sl, initial=0.0,
                        op0=mm, op1=add)
                nc.vector.memset(sat[:, N:SATW], 0.0)
                # gather 4 corner blocks for this image's ROIs
                g = gpool.tile([128, 4 * rb], f32, tag="g")
                nc.gpsimd.ap_gather(g[:], sat[:], idxs[img][:],
                                    channels=128, num_elems=SATW, d=1,
                                    num_idxs=4 * rb)
                win = wpool.tile([128, rb], f32, tag="win")
                tmp = wpool.tile([128, rb], f32, tag="tmp")
                nc.vector.tensor_tensor(win[:], g[:, 0:rb], g[:, rb:2 * rb], op=sub)
                nc.vector.tensor_tensor(tmp[:], g[:, 2 * rb:3 * rb],
                                        g[:, 3 * rb:4 * rb], op=sub)
                nc.vector.tensor_tensor(win[:], win[:], tmp[:], op=sub)
                nc.vector.tensor_tensor(
                    win[:], win[:], recip[:, offs[img]:offs[img] + rb], op=mm)
                if cb == 1 and img == 3:
                    nc.sync.dma_start(dbgsat_d[:], sat[:])
                if cb == 1:
                    nc.sync.dma_start(dbgwin_d[:, offs[img]:offs[img] + rb], win[:])
                # accumulate W1a.T @ roi_mean into the image's column range
                o = offs[img]
                done = 0
                while done < rb:
                    # one matmul may not cross a 512-float PSUM bank boundary
                    nchunk = min(512 - ((o + done) % 512), rb - done)
                    nc.tensor.matmul(psum_main[:, o + done:o + done + nchunk],
                                     w1a[cb][:], win[:, done:done + nchunk],
                                     start=(cb == 0), stop=(cb == 1))
                    done += nchunk
                # global-context column: total sum is the last SAT element
                gcol = wpool.tile([128, 1], f32, tag="gcol")
                nc.scalar.mul(gcol[:], sat[:, N - 1:N], 1.0 / N)
                nc.tensor.matmul(psum_g[:, img:img + 1], w1g[cb][:], gcol[:],
                                 start=(cb == 0), stop=(cb == 1))

        # epilogue: per-ROI global-context term, AllReduce, MLP
        gsb = epool.tile([64, 8], f32)
        nc.vector.memset(gsb[:], 0.0)
        nc.scalar.copy(gsb[:, 0:B], psum_g[:])
        gx = epool.tile([64, rp], f32, tag="gx")
        nc.gpsimd.ap_gather(gx[:], gsb[:], idxg[:], channels=64, num_elems=8,
                            d=1, num_idxs=rp)
        pre = epool.tile([64, rp], f32, tag="pre")
        nc.sync.dma_start(dbggx_d[:], gx[:])
        dbgpm_sb = epool.tile([64, rp], f32, tag="dbgpm", name="dbgpm_sb")
        nc.scalar.copy(dbgpm_sb[:], psum_main[:])
        nc.sync.dma_start(dbgpm_d[:], dbgpm_sb[:])
        nc.vector.tensor_tensor(pre[:], psum_main[:], gx[:], op=add)
        nc.sync.dma_start(dbgpre_d[:], pre[:])
        cc_in = dram.tile([64, rp], f32)
        cc_out = dram.tile([64, rp], f32)
        nc.sync.dma_start(cc_in[:], pre[:])
        nc.gpsimd.collective_compute(
            "AllReduce", add, replica_groups=[list(range(NCORES))],
            ins=[cc_in.opt()], outs=[cc_out.opt()])
        ar = epool.tile([64, rp], f32, tag="ar")
        nc.sync.dma_start(ar[:], cc_out[:])
        nc.scalar.activation(ar[:], ar[:], mybir.ActivationFunctionType.Tanh,
                             bias=b1t[:])
        outsb = epool.tile([1, rp], f32, tag="outsb")
        done = 0
        while done < rp:
            nchunk = min(512, rp - done)
            ps2 = pw2.tile([1, 512], f32, tag="ps2")
            nc.tensor.matmul(ps2[:, 0:nchunk], w2t[:], ar[:, done:done + nchunk],
                             start=True, stop=True)
            # softplus(x) = ln(1 + exp(x)); |x| < ~0.3 so no overflow concerns
            sl = outsb[:, done:done + nchunk]
            nc.scalar.activation(sl, ps2[:, 0:nchunk],
                                 mybir.ActivationFunctionType.Exp, bias=b2t[:])
            nc.scalar.activation(sl, sl, mybir.ActivationFunctionType.Ln,
                                 bias=1.0)
            done += nchunk
        nc.sync.dma_start(out_d[:], outsb[:])
    nc.compile()
    return nc


_CACHE = {}


def _get_program(rbp, offs, rp):
    key = (tuple(rbp), rp)
    if key not in _CACHE:
        _CACHE[key] = _build(rbp, offs, rp)
    return _CACHE[key]


def kernel(features, rois, W1, b1, W2, b2, _want_trace=False, _trace_kwargs=None):
    features = np.ascontiguousarray(np.asarray(features, np.float32))
    W1 = np.asarray(W1, np.float32)
    idx_imgs, idx_g, recip_rep, rbp, offs, rp, colmap = _host_prep(rois)
    nc = _get_program(rbp, offs, rp)

    b1c = np.asarray(b1, np.float32).reshape(64, 1)
    w2tc = np.asarray(W2, np.float32).reshape(1, 64).T.copy()
    b2c = np.asarray(b2, np.float32).reshape(1, 1)
    in_maps = []
    for k in range(NCORES):
        cs = k * CPC
        feat_k = features[:, cs:cs + CPC].reshape(B * CPC, N)
        in_maps.append({
            "feat": np.ascontiguousarray(feat_k),
            "w1a": np.ascontiguousarray(W1[:, cs:cs + CPC].T),
            "w1g": np.ascontiguousarray(W1[:, C + cs:C + cs + CPC].T),
            "recip": recip_rep,
            **{f"idx{img}": idx_imgs[img] for img in range(B)},
            "idxg": idx_g,
            "b1": b1c, "w2t": w2tc, "b2": b2c,
        })
    res = run_bass_kernel_spmd(nc, in_maps, list(range(NCORES)),
                               trace=_want_trace, **(_trace_kwargs or {}))
    scores = res.results[0]["out"][0]          # [rp]
    out = scores[colmap][None, :].astype(np.float32)
    if _want_trace:
        return out, res
    return out

